# revision 28
# baseline (speedup 1.0000x reference)
"""EnhancedGDN Trainium2 kernel (v3 — gather-free).

Data-parallel over batch B=64 across 8 NeuronCores (8 graphs each).
All 64 graphs share one edge list; the only gpsimd work is ONE shared
cnt-mask scatter (ap_gather is ~36us/call on HW — avoided entirely).

Per device (8 graphs):
  t_out = data @ v_w.T + v_b              (temporal attn: window=1)
  x     = data @ lin_w.T                  (xnm tiles, node-major)
  s_i/s_j per node from att vectors (+ tiled emb scores)
  lnc   = Ln(scatter(cnt))                (once; ln(0)=-inf kills non-edges)
  per graph g:
    alpha[j,i] = s_j[g,j] + s_i[g,i]      (DVE: rank-1 siB + per-partition sj)
    W = exp(lrelu(alpha) + lnc)           (dense [src, dst], f16)
    den = ones @ W ; rdenB = recip(ones x den)   (ACT table recip)
    agg = (xnm @ W) * rdenB               (normalize on evict)
  BatchNorm over all 64k nodes (AllReduce of sums) + ReLU
  out   = (relu([s_out|t_out] @ f_w1.T + f_b1) @ f_w2.T + f_b2) @ out_w.T + out_b
"""

import os

os.environ.setdefault("NEURON_RT_RESET_CORES", "1")

import numpy as np

import concourse.bass as bass
import concourse.bacc as bacc
import concourse.tile as tile
from concourse import mybir
from concourse.bass_utils import run_bass_kernel_spmd

B, N, D, E = 64, 1000, 128, 20000
M = 8          # devices
G = B // M     # graphs per device
NG = G * N     # nodes per device
NEG = 0.2
EPS = 1e-5

F16 = mybir.dt.float16
F32 = mybir.dt.float32
I16 = mybir.dt.int16
AF = mybir.ActivationFunctionType
ALU = mybir.AluOpType

_CACHE = {}


# ---------------------------------------------------------------- host index prep
def _prep_indices(edge_index):
    src = edge_index[0].astype(np.int64)
    dst = edge_index[1].astype(np.int64)
    key = dst * N + src
    uniq, cnt = np.unique(key, return_counts=True)
    ii = uniq // N
    jj = uniq % N
    # add self loops (reference removes none exist, then adds them)
    ii = np.concatenate([ii, np.arange(N)])
    jj = np.concatenate([jj, np.arange(N)])
    cc = np.concatenate([cnt, np.ones(N, np.int64)]).astype(np.float32)

    # out-CSR grouped by src j: scatter indices + cnt values, [j, (c, k)]
    order = np.argsort(jj, kind="stable")
    oj, oi, oc = jj[order], ii[order], cc[order]
    outdeg = np.bincount(oj, minlength=N)
    Kout = max(4, (int(outdeg.max()) + 3) // 4 * 4)
    out_i_sc = np.full((1024, Kout), -1, np.int64)  # -1 pads ignored by scatter
    out_c = np.zeros((1024, Kout), np.float32)
    starts = np.searchsorted(oj, np.arange(N))
    kpos = np.arange(len(oj)) - starts[oj]
    out_i_sc[oj, kpos] = oi
    out_c[oj, kpos] = oc

    scat = out_i_sc.reshape(8, 128, Kout).transpose(1, 0, 2)      # [j, c, k]
    sidx2 = np.ascontiguousarray(scat.reshape(128, 8 * Kout)).astype(np.int16)
    cvals = out_c.reshape(8, 128, Kout).transpose(1, 0, 2)
    cntw = np.ascontiguousarray(cvals.reshape(128, 8 * Kout)).astype(np.float16)
    return dict(Kout=Kout, sidx2=sidx2, cntw=cntw)


# ---------------------------------------------------------------- device module
def _build(Kout, n_cores):
    NH = 16             # chunks of 500 over 8000
    CH = 500

    nc = bacc.Bacc("TRN2", target_bir_lowering=False, debug=False,
                   num_devices=n_cores)

    def din(name, shape, dt):
        return nc.dram_tensor(name, shape, dt, kind="ExternalInput").ap()

    x0T = din("x0T", [128, NG], F16)
    embT = din("embT", [128, N], F16)
    wpack = din("wpack", [128, 1184], F16)
    bpack = din("bpack", [128, 8], F32)
    outb = din("outb", [1, 1], F32)
    sidx2_d = din("sidx2", [128, 8 * Kout], I16)
    cntw_d = din("cntw", [128, 8 * Kout], F16)
    y_out = nc.dram_tensor("y", [1, NG], F32, kind="ExternalOutput").ap()

    cc_in = nc.dram_tensor("cc_in", [128, 2], F32).ap()
    cc_out = nc.dram_tensor("cc_out", [128, 2], F32, addr_space="Shared").ap()
    cc_win = nc.dram_tensor("cc_win", [128, 2], F32).ap()
    cc_wout = nc.dram_tensor("cc_wout", [128, 2], F32, addr_space="Shared").ap()

    # wpack column layout
    W_LIN, W_V, W_F2, W_F1, W_ATTQ, W_ATTEM, W_OUT, W_ID, W_ONE = (
        0, 128, 256, 384, 640, 642, 644, 645, 773)
    W_ONER = 774
    W_F2P = 902
    W_LINP = 1030
    # bpack columns: v_b, gnn_bias, f_b1, f_b2, bn_gamma, bn_beta
    B_VB, B_GNN, B_FB1, B_FB2, B_GAM, B_BET, B_EPS = 0, 1, 2, 3, 4, 5, 6

    with tile.TileContext(nc) as tc:
        with (
            tc.tile_pool(name="cst", bufs=1) as cst,
            tc.tile_pool(name="big", bufs=1) as big,
            tc.tile_pool(name="wt", bufs=3) as wtp,
            tc.tile_pool(name="sm", bufs=1) as sm,
            tc.tile_pool(name="pg", bufs=2) as pg,
            tc.tile_pool(name="stg", bufs=4) as stg,
            tc.tile_pool(name="psS", bufs=2, space="PSUM") as psS,
            tc.tile_pool(name="psX", bufs=2, space="PSUM") as psX,
            tc.tile_pool(name="psA", bufs=4, space="PSUM") as psA,
        ):
            # ---- load constants (x0 first: biggest + on critical path)
            x0 = big.tile([128, NG], F16, tag="s16a")
            nc.sync.dma_start(x0[:], x0T)
            wp = cst.tile([128, 1184], F16)
            nc.sync.dma_start(wp[:], wpack)
            bp = cst.tile([128, 8], F32)
            nc.sync.dma_start(bp[:], bpack)
            ob = cst.tile([1, 1], F32)
            nc.sync.dma_start(ob[:], outb)
            emb = cst.tile([128, N], F16)
            nc.sync.dma_start(emb[:], embT)
            sidx = cst.tile([128, 8 * Kout], I16)
            nc.sync.dma_start(sidx[:], sidx2_d)
            cntw = cst.tile([128, 8 * Kout], F16)
            nc.sync.dma_start(cntw[:], cntw_d)

            def bias(col):
                return bp[:, col:col + 1]

            # warm up the collective path early (absorbs setup/skew)
            warm = sm.tile([128, 2], F32)
            nc.vector.memset(warm[:], 0.0)
            nc.sync.dma_start(cc_win, warm[:])
            nc.gpsimd.collective_compute(
                "AllReduce", ALU.add,
                replica_groups=[list(range(n_cores))],
                ins=[cc_win], outs=[cc_wout])

            # ---- shared cnt mask -> lnc = Ln(cnt-mask), once for all graphs
            lnc = big.tile([128, NG], F16, tag="lnc")
            for t in range(8):
                nc.gpsimd.local_scatter(
                    lnc[:, t * 1000:(t + 1) * 1000],
                    cntw[:, t * Kout:(t + 1) * Kout],
                    sidx[:, t * Kout:(t + 1) * Kout],
                    channels=128, num_elems=N, num_idxs=Kout)
            for q in range(4):
                nc.scalar.activation(lnc[:, q * 2000:(q + 1) * 2000],
                                     lnc[:, q * 2000:(q + 1) * 2000], AF.Ln)

            # ---- node scores: attc = lin^T attq ; emsc = att_em @ emb
            attc = sm.tile([128, 2], F16)
            pat = psX.tile([128, 128], F32, tag="X")
            nc.tensor.matmul(pat[:, 0:2], wp[:, W_LINP:W_LINP + 128],
                             wp[:, W_ATTQ:W_ATTQ + 2], start=True, stop=True)
            nc.vector.tensor_copy(attc[:], pat[:, 0:2])
            emsc = sm.tile([2, N], F32)
            for h in range(2):
                ps = psS.tile([2, CH], F32, tag="S")
                nc.tensor.matmul(ps[:], wp[:, W_ATTEM:W_ATTEM + 2],
                                 emb[:, h * CH:(h + 1) * CH], start=True, stop=True)
                nc.vector.tensor_copy(emsc[:, h * CH:(h + 1) * CH], ps[:])

            # scores -> sNN [16, 1024] f16 (rows 0-7 s_i[g], rows 8-15 s_j[g])
            sNN = sm.tile([16, 1024], F16)
            nc.vector.memset(sNN[:], 0.0)
            for h in range(NH):
                s = h * CH
                g, off = divmod(s, 1000)
                ps = psS.tile([2, CH], F32, tag="S")
                nc.tensor.matmul(ps[:], attc[:, 0:2],
                                 x0[:, s:s + CH], start=True, stop=True)
                st = stg.tile([2, CH], F16, tag="sc")
                nc.vector.tensor_tensor(st[:], ps[:], emsc[:, off:off + CH],
                                        op=ALU.add)
                nc.sync.dma_start(
                    sNN[:, off:off + CH].rearrange("(a g) f -> g a f", a=2)[g],
                    st[:, :])

            # ---- sAB[j, 16t+r] = sNN[r, 128t+j] (transposes; col 16t+8+g = s_j)
            sAB = sm.tile([128, 128], F32)
            for t in range(8):
                pt = psX.tile([128, 128], F16, tag="X")
                nc.tensor.transpose(pt[:, 0:16], sNN[:, t * 128:(t + 1) * 128],
                                    wp[0:16, W_ID:W_ID + 16])
                nc.vector.tensor_copy(sAB[:, t * 16:(t + 1) * 16], pt[:, 0:16])
            # siR row0: s_i[g] rows concatenated at g*1024
            siR = sm.tile([1, 8 * 1024], F16)
            for g in range(G):
                nc.sync.dma_start(siR[0:1, g * 1024:g * 1024 + 1024],
                                  sNN[g:g + 1, :])

            # ---- xnm tiles (x in node-major, direct from x0) + tT (off path)
            xnm = big.tile([128, 64 * 128], F16, tag="xnm")
            for g in range(G):
                for t in range(8):
                    w = 128 if t < 7 else 104
                    n0 = g * 1000 + t * 128
                    px = psX.tile([128, 128], F32, tag="X")
                    nc.tensor.matmul(px[0:w, :], x0[:, n0:n0 + w],
                                     wp[:, W_LIN:W_LIN + 128], start=True,
                                     stop=True)
                    dst = xnm[0:w, (g * 8 + t) * 128:(g * 8 + t) * 128 + 128]
                    if t % 2 == 0:
                        nc.scalar.activation(dst, px[0:w, :], AF.Identity)
                    else:
                        nc.vector.tensor_copy(dst, px[0:w, :])
            tT = big.tile([128, NG], F16, tag="tT")
            for h in range(NH):
                s = h * CH
                ps2 = psA.tile([128, CH], F32, tag="A")
                nc.tensor.matmul(ps2[:], wp[:, W_V:W_V + 128],
                                 x0[:, s:s + CH], start=True, stop=True)
                if h % 2 == 0:
                    nc.scalar.activation(tT[:, s:s + CH], ps2[:], AF.Identity,
                                         bias=bias(B_VB))
                else:
                    nc.vector.tensor_scalar(tT[:, s:s + CH], ps2[:], bias(B_VB),
                                            None, op0=ALU.add)

            # ---- per-graph: dense W on compute engines, den+agg matmuls
            aggT = big.tile([128, NG], F16, tag="agg")
            sumP = sm.tile([128, 8], F32)
            sqP = sm.tile([128, 8], F32)
            for g in range(G):
                # siB = ones (x) s_i[g]  [128, 1000]
                siB = pg.tile([128, 1024], F16, tag="siB")
                for hf in range(2):
                    pe = psA.tile([128, CH], F32, tag="A")
                    nc.tensor.matmul(pe[:], wp[0:1, W_ONER:W_ONER + 128],
                                     siR[0:1, g * 1024 + hf * CH:
                                         g * 1024 + hf * CH + CH],
                                     start=True, stop=True)
                    nc.vector.tensor_copy(siB[:, hf * CH:hf * CH + CH], pe[:])
                # W = exp(lrelu(siB + s_j) + lnc), tile by src chunk t
                WT = wtp.tile([128, 8000], F16, tag="wt")
                for t in range(8):
                    sl = slice(t * 1000, (t + 1) * 1000)
                    sjb = sAB[:, 16 * t + 8 + g:16 * t + 8 + g + 1]
                    tmp = pg.tile([128, 1000], F16, tag="tmp")
                    nc.vector.tensor_scalar(tmp[:], siB[:, 0:1000], sjb, None,
                                            op0=ALU.add)
                    nc.vector.scalar_tensor_tensor(tmp[:], tmp[:], NEG, tmp[:],
                                                   op0=ALU.mult, op1=ALU.max)
                    nc.vector.tensor_tensor(tmp[:], tmp[:], lnc[:, sl],
                                            op=ALU.add)
                    nc.scalar.activation(WT[:, sl], tmp[:], AF.Exp)
                # den row -> ln(den) -> broadcast -> exp(-x)  (1/den via tables)
                denR = pg.tile([2, 1024], F16, tag="denR")
                for hf in range(2):
                    pd = psS.tile([2, CH], F32, tag="S")
                    for t in range(8):
                        kt = 128 if t < 7 else 104
                        nc.tensor.matmul(
                            pd[0:1, :], wp[0:kt, W_ONE:W_ONE + 1],
                            WT[0:kt, t * 1000 + hf * CH:t * 1000 + hf * CH + CH],
                            start=(t == 0), stop=(t == 7))
                    nc.scalar.activation(denR[0:1, hf * CH:hf * CH + CH],
                                         pd[0:1, :], AF.Ln)
                rdenB = pg.tile([128, 1024], F16, tag="rdenB")
                for hf in range(2):
                    pr = psA.tile([128, CH], F32, tag="A")
                    nc.tensor.matmul(pr[:], wp[0:1, W_ONER:W_ONER + 128],
                                     denR[0:1, hf * CH:hf * CH + CH],
                                     start=True, stop=True)
                    nc.scalar.activation(rdenB[:, hf * CH:hf * CH + CH], pr[:],
                                         AF.Exp, scale=-1.0)
                # agg = (xnm @ W) * rdenB
                for hf in range(2):
                    pa = psA.tile([128, CH], F32, tag="A")
                    for t in range(8):
                        kt = 128 if t < 7 else 104
                        nc.tensor.matmul(
                            pa[:], xnm[0:kt, (g * 8 + t) * 128:
                                       (g * 8 + t) * 128 + 128],
                            WT[0:kt, t * 1000 + hf * CH:t * 1000 + hf * CH + CH],
                            start=(t == 0), stop=(t == 7))
                    nc.vector.scalar_tensor_tensor(
                        aggT[:, g * 1000 + hf * CH:g * 1000 + hf * CH + CH],
                        pa[:], 1.0, rdenB[:, hf * CH:hf * CH + CH],
                        op0=ALU.mult, op1=ALU.mult)
                # BN partial sums on DVE
                asl = aggT[:, g * 1000:(g + 1) * 1000]
                nc.vector.tensor_reduce(sumP[:, g:g + 1], asl,
                                        axis=mybir.AxisListType.X, op=ALU.add)
                sqs = pg.tile([128, 1000], F16, tag="sqs")
                nc.vector.scalar_tensor_tensor(sqs[:], asl, 1.0, asl,
                                               op0=ALU.mult, op1=ALU.mult)
                nc.vector.tensor_reduce(sqP[:, g:g + 1], sqs[:],
                                        axis=mybir.AxisListType.X, op=ALU.add)

            # ---- BN stats (gnn_bias folded analytically) + AllReduce
            stats = sm.tile([128, 2], F32)
            s1u = sm.tile([128, 4], F32)
            nc.vector.tensor_reduce(s1u[:, 0:1], sumP[:],
                                    axis=mybir.AxisListType.X, op=ALU.add)
            nc.vector.tensor_reduce(s1u[:, 1:2], sqP[:],
                                    axis=mybir.AxisListType.X, op=ALU.add)
            gb = bias(B_GNN)
            nc.vector.tensor_scalar(s1u[:, 2:3], gb, float(B * N), None,
                                    op0=ALU.mult)
            nc.vector.tensor_tensor(stats[:, 0:1], s1u[:, 0:1], s1u[:, 2:3],
                                    op=ALU.add)
            nc.vector.scalar_tensor_tensor(stats[:, 1:2], s1u[:, 0:1], 2.0,
                                           s1u[:, 2:3], op0=ALU.mult, op1=ALU.add)
            nc.vector.tensor_tensor(stats[:, 1:2], stats[:, 1:2], gb, op=ALU.mult)
            nc.vector.tensor_tensor(stats[:, 1:2], stats[:, 1:2], s1u[:, 1:2],
                                    op=ALU.add)
            nc.sync.dma_start(cc_in, stats[:])
            nc.gpsimd.collective_compute(
                "AllReduce", ALU.add,
                replica_groups=[list(range(n_cores))],
                ins=[cc_in], outs=[cc_out])
            gstats = sm.tile([128, 2], F32)
            nc.sync.dma_start(gstats[:], cc_out)

            # ---- BN coefficients A, Bv
            cf = sm.tile([128, 8], F32)
            mu, msq, var, rsd, A_, Bv = (cf[:, i:i + 1] for i in range(6))
            inv_n = 1.0 / (B * N)
            nc.vector.tensor_scalar_mul(mu, gstats[:, 0:1], inv_n)
            nc.vector.tensor_scalar_mul(msq, gstats[:, 1:2], inv_n)
            nc.vector.tensor_tensor(var, mu, mu, op=ALU.mult)
            nc.vector.tensor_sub(var, msq, var)
            nc.scalar.activation(var, var, AF.Sqrt, bias=bias(B_EPS))
            nc.vector.reciprocal(rsd, var)
            nc.vector.tensor_tensor(A_, bias(B_GAM), rsd, op=ALU.mult)
            nc.vector.tensor_tensor(Bv, mu, A_, op=ALU.mult)
            nc.vector.tensor_sub(Bv, bias(B_BET), Bv)
            nc.vector.tensor_tensor(cf[:, 6:7], bias(B_GNN), A_, op=ALU.mult)
            nc.vector.tensor_tensor(Bv, Bv, cf[:, 6:7], op=ALU.add)

            # ---- s_out = relu(A*agg + Bv) (in place; split ACT/DVE)
            HF2 = NG // 2
            nc.scalar.activation(aggT[:, 0:HF2], aggT[:, 0:HF2], AF.Relu,
                                 bias=Bv, scale=A_)
            nc.vector.tensor_scalar(aggT[:, HF2:NG], aggT[:, HF2:NG], A_, Bv,
                                    op0=ALU.mult, op1=ALU.add)
            nc.vector.tensor_scalar_max(aggT[:, HF2:NG], aggT[:, HF2:NG], 0.0)

            # ---- fusion MLP + out layer
            hT = big.tile([128, NG], F16, tag="s16a")
            for h in range(NH):
                s = h * CH
                ps = psA.tile([128, CH], F32, tag="A")
                nc.tensor.matmul(ps[:], wp[:, W_F1:W_F1 + 128],
                                 aggT[:, s:s + CH], start=True, stop=False)
                nc.tensor.matmul(ps[:], wp[:, W_F1 + 128:W_F1 + 256],
                                 tT[:, s:s + CH], start=False, stop=True)
                if h % 2 == 0:
                    nc.scalar.activation(hT[:, s:s + CH], ps[:], AF.Relu,
                                         bias=bias(B_FB1))
                else:
                    nc.vector.tensor_scalar(hT[:, s:s + CH], ps[:], bias(B_FB1),
                                            None, op0=ALU.add)
                    nc.vector.tensor_scalar_max(hT[:, s:s + CH], hT[:, s:s + CH],
                                                0.0)
            # composite head: c = f_w2 @ out_w ; cb = <out_w, f_b2> + out_b
            cvec = sm.tile([128, 2], F16)
            cb = sm.tile([1, 2], F32)
            nc.vector.tensor_copy(cvec[:, 1:2], bias(B_FB2))  # f_b2 -> f16
            pc = psS.tile([2, CH], F32, tag="S")
            nc.tensor.matmul(pc[0:1, 0:1], cvec[:, 1:2],
                             wp[:, W_OUT:W_OUT + 1], start=True, stop=True)
            pc2 = psX.tile([128, 128], F32, tag="X")
            nc.tensor.matmul(pc2[:, 0:1], wp[:, W_F2P:W_F2P + 128],
                             wp[:, W_OUT:W_OUT + 1], start=True, stop=True)
            nc.vector.tensor_copy(cvec[:, 0:1], pc2[:, 0:1])
            nc.vector.tensor_copy(cb[:, 0:1], pc[0:1, 0:1])
            nc.vector.tensor_tensor(cb[:, 1:2], cb[:, 0:1], ob[:], op=ALU.add)
            for h in range(NH):
                s = h * CH
                ps = psS.tile([2, CH], F32, tag="S")
                nc.tensor.matmul(ps[0:1, :], cvec[:, 0:1],
                                 hT[:, s:s + CH], start=True, stop=True)
                yst = stg.tile([2, CH], F32, tag="ys")
                nc.scalar.activation(yst[0:1, :], ps[0:1, :],
                                     AF.Identity, bias=cb[:, 1:2])
                nc.sync.dma_start(y_out[:, s:s + CH], yst[0:1, :])

    nc.compile()
    return nc


# ---------------------------------------------------------------- entry point
def _prepare(inputs):
    """Returns (nc, in_maps) — host prep + cached module build."""
    data = np.asarray(inputs["data"], np.float32)
    edge_index = np.asarray(inputs["edge_index"])

    pre = _prep_indices(edge_index)
    Kout = pre["Kout"]

    key = (Kout,)
    if key not in _CACHE:
        _CACHE[key] = _build(Kout, M)
    nc = _CACHE[key]

    f16 = np.float16

    def t16(a):  # transpose [r, c] -> [c, r] f16 contiguous
        return np.ascontiguousarray(np.asarray(a, np.float32).T).astype(f16)

    wpack = np.zeros((128, 1184), f16)
    wpack[:, 0:128] = t16(inputs["lin_w"])
    wpack[:, 128:256] = t16(inputs["v_w"])
    wpack[:, 256:384] = t16(inputs["f_w2"])
    wpack[:, 384:640] = np.ascontiguousarray(
        np.asarray(inputs["f_w1"], np.float32).T).astype(f16).reshape(2, 128, 128
        ).transpose(1, 0, 2).reshape(128, 256)
    wpack[:, 640] = np.asarray(inputs["att_i"], np.float32).astype(f16)
    wpack[:, 641] = np.asarray(inputs["att_j"], np.float32).astype(f16)
    wpack[:, 642] = np.asarray(inputs["att_em_i"], np.float32).astype(f16)
    wpack[:, 643] = np.asarray(inputs["att_em_j"], np.float32).astype(f16)
    wpack[:, 644] = np.asarray(inputs["out_w"], np.float32)[0].astype(f16)
    wpack[:, 645:773] = np.eye(128, dtype=f16)
    wpack[:, 773] = 1.0
    wpack[0, 774:902] = 1.0
    wpack[:, 902:1030] = np.asarray(inputs["f_w2"], np.float32).astype(f16)
    wpack[:, 1030:1158] = np.asarray(inputs["lin_w"], np.float32).astype(f16)

    bpack = np.zeros((128, 8), np.float32)
    bpack[:, 0] = np.asarray(inputs["v_b"], np.float32)
    bpack[:, 1] = np.asarray(inputs["gnn_bias"], np.float32)
    bpack[:, 2] = np.asarray(inputs["f_b1"], np.float32)
    bpack[:, 3] = np.asarray(inputs["f_b2"], np.float32)
    bpack[:, 4] = np.asarray(inputs["bn_gamma"], np.float32)
    bpack[:, 5] = np.asarray(inputs["bn_beta"], np.float32)
    bpack[:, 6] = EPS
    outb = np.asarray(inputs["out_b"], np.float32).reshape(1, 1)

    embT = t16(inputs["emb"])

    shared = dict(
        embT=embT, wpack=wpack, bpack=bpack, outb=outb,
        sidx2=pre["sidx2"], cntw=pre["cntw"],
    )
    in_maps = []
    for d in range(M):
        x0T = np.ascontiguousarray(
            data[d * G:(d + 1) * G].transpose(2, 0, 1).reshape(128, NG)
        ).astype(f16)
        in_maps.append(dict(shared, x0T=x0T))
    return nc, in_maps


def kernel(**inputs):
    nc, in_maps = _prepare(inputs)
    res = run_bass_kernel_spmd(nc, in_maps, list(range(M)))
    out = np.empty(B * N, np.float32)
    for d in range(M):
        out[d * NG:(d + 1) * NG] = res.results[d]["y"].reshape(-1)
    return out


# revision 32
# speedup vs baseline: 1.1322x; 1.1322x over previous
"""EnhancedGDN Trainium2 kernel (v3 — gather-free).

Data-parallel over batch B=64 across 8 NeuronCores (8 graphs each).
All 64 graphs share one edge list; the only gpsimd work is ONE shared
cnt-mask scatter (ap_gather is ~36us/call on HW — avoided entirely).

Per device (8 graphs):
  t_out = data @ v_w.T + v_b              (temporal attn: window=1)
  x     = data @ lin_w.T                  (xnm tiles, node-major)
  s_i/s_j per node from att vectors (+ tiled emb scores)
  lnc   = Ln(scatter(cnt))                (once; ln(0)=-inf kills non-edges)
  per graph g:
    alpha[j,i] = s_j[g,j] + s_i[g,i]      (DVE: rank-1 siB + per-partition sj)
    W = exp(lrelu(alpha) + lnc)           (dense [src, dst], f16)
    den = ones @ W ; rdenB = recip(ones x den)   (ACT table recip)
    agg = (xnm @ W) * rdenB               (normalize on evict)
  BatchNorm over all 64k nodes (AllReduce of sums) + ReLU
  out   = (relu([s_out|t_out] @ f_w1.T + f_b1) @ f_w2.T + f_b2) @ out_w.T + out_b
"""

import os

os.environ.setdefault("NEURON_RT_RESET_CORES", "1")

import numpy as np

import concourse.bass as bass
import concourse.bacc as bacc
import concourse.tile as tile
from concourse import mybir
from concourse.bass_utils import run_bass_kernel_spmd

B, N, D, E = 64, 1000, 128, 20000
M = 8          # devices
G = B // M     # graphs per device
NG = G * N     # nodes per device
NEG = 0.2
EPS = 1e-5

F16 = mybir.dt.float16
F32 = mybir.dt.float32
I16 = mybir.dt.int16
AF = mybir.ActivationFunctionType
ALU = mybir.AluOpType

_CACHE = {}


# ---------------------------------------------------------------- host index prep
def _prep_indices(edge_index):
    src = edge_index[0].astype(np.int64)
    dst = edge_index[1].astype(np.int64)
    key = dst * N + src
    uniq, cnt = np.unique(key, return_counts=True)
    ii = uniq // N
    jj = uniq % N
    # add self loops (reference removes none exist, then adds them)
    ii = np.concatenate([ii, np.arange(N)])
    jj = np.concatenate([jj, np.arange(N)])
    cc = np.concatenate([cnt, np.ones(N, np.int64)]).astype(np.float32)

    # out-CSR grouped by src j: scatter indices + cnt values, [j, (c, k)]
    order = np.argsort(jj, kind="stable")
    oj, oi, oc = jj[order], ii[order], cc[order]
    outdeg = np.bincount(oj, minlength=N)
    Kout = max(4, (int(outdeg.max()) + 3) // 4 * 4)
    out_i_sc = np.full((1024, Kout), -1, np.int64)  # -1 pads ignored by scatter
    out_c = np.zeros((1024, Kout), np.float32)
    starts = np.searchsorted(oj, np.arange(N))
    kpos = np.arange(len(oj)) - starts[oj]
    out_i_sc[oj, kpos] = oi
    out_c[oj, kpos] = oc

    scat = out_i_sc.reshape(8, 128, Kout).transpose(1, 0, 2)      # [j, c, k]
    sidx2 = np.ascontiguousarray(scat.reshape(128, 8 * Kout)).astype(np.int16)
    cvals = out_c.reshape(8, 128, Kout).transpose(1, 0, 2)
    cntw = np.ascontiguousarray(cvals.reshape(128, 8 * Kout)).astype(np.float16)
    return dict(Kout=Kout, sidx2=sidx2, cntw=cntw)


# ---------------------------------------------------------------- device module
def _build(Kout, n_cores):
    NH = 16             # chunks of 500 over 8000
    CH = 500

    nc = bacc.Bacc("TRN2", target_bir_lowering=False, debug=False,
                   num_devices=n_cores)

    def din(name, shape, dt):
        return nc.dram_tensor(name, shape, dt, kind="ExternalInput").ap()

    x0T = din("x0T", [128, NG], F16)
    embT = din("embT", [128, N], F16)
    wpack = din("wpack", [128, 1184], F16)
    bpack = din("bpack", [128, 8], F32)
    outb = din("outb", [1, 1], F32)
    sidx2_d = din("sidx2", [128, 8 * Kout], I16)
    cntw_d = din("cntw", [128, 8 * Kout], F16)
    y_out = nc.dram_tensor("y", [1, NG], F32, kind="ExternalOutput").ap()

    cc_in = nc.dram_tensor("cc_in", [128, 2], F32).ap()
    cc_out = nc.dram_tensor("cc_out", [128, 2], F32, addr_space="Shared").ap()
    cc_win = nc.dram_tensor("cc_win", [128, 2], F32).ap()
    cc_wout = nc.dram_tensor("cc_wout", [128, 2], F32, addr_space="Shared").ap()

    # wpack column layout
    W_LIN, W_V, W_F2, W_F1, W_ATTQ, W_ATTEM, W_OUT, W_ID, W_ONE = (
        0, 128, 256, 384, 640, 642, 644, 645, 773)
    W_ONER = 774
    W_F2P = 902
    W_LINP = 1030
    # bpack columns: v_b, gnn_bias, f_b1, f_b2, bn_gamma, bn_beta
    B_VB, B_GNN, B_FB1, B_FB2, B_GAM, B_BET, B_EPS = 0, 1, 2, 3, 4, 5, 6

    with tile.TileContext(nc) as tc:
        with (
            tc.tile_pool(name="cst", bufs=1) as cst,
            tc.tile_pool(name="big", bufs=1) as big,
            tc.tile_pool(name="wt", bufs=3) as wtp,
            tc.tile_pool(name="sm", bufs=1) as sm,
            tc.tile_pool(name="pg", bufs=2) as pg,
            tc.tile_pool(name="stg", bufs=4) as stg,
            tc.tile_pool(name="psS", bufs=2, space="PSUM") as psS,
            tc.tile_pool(name="psX", bufs=2, space="PSUM") as psX,
            tc.tile_pool(name="psA", bufs=4, space="PSUM") as psA,
        ):
            # ---- load constants (x0 first: biggest + on critical path)
            x0 = big.tile([128, NG], F16, tag="s16a")
            nc.sync.dma_start(x0[:], x0T)
            wp = cst.tile([128, 1184], F16)
            nc.sync.dma_start(wp[:], wpack)
            bp = cst.tile([128, 8], F32)
            nc.sync.dma_start(bp[:], bpack)
            ob = cst.tile([1, 1], F32)
            nc.sync.dma_start(ob[:], outb)
            emb = cst.tile([128, N], F16)
            nc.sync.dma_start(emb[:], embT)
            sidx = cst.tile([128, 8 * Kout], I16)
            nc.sync.dma_start(sidx[:], sidx2_d)
            cntw = cst.tile([128, 8 * Kout], F16)
            nc.sync.dma_start(cntw[:], cntw_d)

            def bias(col):
                return bp[:, col:col + 1]

            # warm up the collective path early (absorbs setup/skew)
            warm = sm.tile([128, 2], F32)
            nc.vector.memset(warm[:], 0.0)
            nc.sync.dma_start(cc_win, warm[:])
            nc.gpsimd.collective_compute(
                "AllReduce", ALU.add,
                replica_groups=[list(range(n_cores))],
                ins=[cc_win], outs=[cc_wout])

            # ---- shared cnt mask -> lnc = Ln(cnt-mask), once for all graphs
            lnc = big.tile([128, NG], F16, tag="lnc")
            for t in range(8):
                nc.gpsimd.local_scatter(
                    lnc[:, t * 1000:(t + 1) * 1000],
                    cntw[:, t * Kout:(t + 1) * Kout],
                    sidx[:, t * Kout:(t + 1) * Kout],
                    channels=128, num_elems=N, num_idxs=Kout)
            for q in range(4):
                nc.scalar.activation(lnc[:, q * 2000:(q + 1) * 2000],
                                     lnc[:, q * 2000:(q + 1) * 2000], AF.Ln)

            # ---- node scores: attc = lin^T attq ; emsc = att_em @ emb
            attc = sm.tile([128, 2], F16)
            pat = psX.tile([128, 128], F32, tag="X")
            nc.tensor.matmul(pat[:, 0:2], wp[:, W_LINP:W_LINP + 128],
                             wp[:, W_ATTQ:W_ATTQ + 2], start=True, stop=True)
            nc.vector.tensor_copy(attc[:], pat[:, 0:2])
            emsc = sm.tile([2, N], F32)
            for h in range(2):
                ps = psS.tile([2, CH], F32, tag="S")
                nc.tensor.matmul(ps[:], wp[:, W_ATTEM:W_ATTEM + 2],
                                 emb[:, h * CH:(h + 1) * CH], start=True, stop=True)
                nc.vector.tensor_copy(emsc[:, h * CH:(h + 1) * CH], ps[:])

            # scores -> sNN [16, 1024] f16 (rows 0-7 s_i[g], rows 8-15 s_j[g])
            sNN = sm.tile([16, 1024], F16)
            nc.vector.memset(sNN[:], 0.0)
            for h in range(NH):
                s = h * CH
                g, off = divmod(s, 1000)
                ps = psS.tile([2, CH], F32, tag="S")
                nc.tensor.matmul(ps[:], attc[:, 0:2],
                                 x0[:, s:s + CH], start=True, stop=True)
                st = stg.tile([2, CH], F16, tag="sc")
                nc.vector.tensor_tensor(st[:], ps[:], emsc[:, off:off + CH],
                                        op=ALU.add)
                nc.sync.dma_start(
                    sNN[:, off:off + CH].rearrange("(a g) f -> g a f", a=2)[g],
                    st[:, :])

            # ---- sAB[j, 16t+r] = sNN[r, 128t+j] (transposes; col 16t+8+g = s_j)
            sAB = sm.tile([128, 128], F32)
            for t in range(8):
                pt = psX.tile([128, 128], F16, tag="X")
                nc.tensor.transpose(pt[:, 0:16], sNN[:, t * 128:(t + 1) * 128],
                                    wp[0:16, W_ID:W_ID + 16])
                nc.vector.tensor_copy(sAB[:, t * 16:(t + 1) * 16], pt[:, 0:16])
            # siR row0: s_i[g] rows concatenated at g*1024
            siR = sm.tile([1, 8 * 1024], F16)
            for g in range(G):
                nc.sync.dma_start(siR[0:1, g * 1024:g * 1024 + 1024],
                                  sNN[g:g + 1, :])

            # ---- xnm tiles (x in node-major, direct from x0) + tT (off path)
            xnm = big.tile([128, 64 * 128], F16, tag="xnm")
            for g in range(G):
                for t in range(8):
                    w = 128 if t < 7 else 104
                    n0 = g * 1000 + t * 128
                    px = psX.tile([128, 128], F32, tag="X")
                    nc.tensor.matmul(px[0:w, :], x0[:, n0:n0 + w],
                                     wp[:, W_LIN:W_LIN + 128], start=True,
                                     stop=True)
                    dst = xnm[0:w, (g * 8 + t) * 128:(g * 8 + t) * 128 + 128]
                    if t % 2 == 0:
                        nc.scalar.activation(dst, px[0:w, :], AF.Identity)
                    else:
                        nc.vector.tensor_copy(dst, px[0:w, :])
            tT = big.tile([128, NG], F16, tag="tT")
            for h in range(NH):
                s = h * CH
                ps2 = psA.tile([128, CH], F32, tag="A")
                nc.tensor.matmul(ps2[:], wp[:, W_V:W_V + 128],
                                 x0[:, s:s + CH], start=True, stop=True)
                if h % 2 == 0:
                    nc.scalar.activation(tT[:, s:s + CH], ps2[:], AF.Identity,
                                         bias=bias(B_VB))
                else:
                    nc.vector.tensor_scalar(tT[:, s:s + CH], ps2[:], bias(B_VB),
                                            None, op0=ALU.add)

            # ---- per-graph: dense W on compute engines, den+agg matmuls
            aggT = big.tile([128, NG], F16, tag="agg")
            sumP = sm.tile([128, 8], F32)
            sqP = sm.tile([128, 8], F32)
            for g in range(G):
                # siB = ones (x) s_i[g]  [128, 1000]
                siB = pg.tile([128, 1024], F16, tag="siB")
                for hf in range(2):
                    pe = psA.tile([128, CH], F32, tag="A")
                    nc.tensor.matmul(pe[:], wp[0:1, W_ONER:W_ONER + 128],
                                     siR[0:1, g * 1024 + hf * CH:
                                         g * 1024 + hf * CH + CH],
                                     start=True, stop=True)
                    if hf == 0:
                        nc.vector.tensor_copy(siB[:, 0:CH], pe[:])
                    else:
                        nc.scalar.activation(siB[:, CH:2 * CH], pe[:],
                                             AF.Identity)
                # W = exp(lrelu(siB + s_j) + lnc), in place on WT, phase-grouped
                # so ACT switches tables only twice per graph
                WT = wtp.tile([128, 8000], F16, tag="wt")
                for t in range(8):
                    sl = slice(t * 1000, (t + 1) * 1000)
                    sjb = sAB[:, 16 * t + 8 + g:16 * t + 8 + g + 1]
                    if t % 2 == 0:
                        nc.scalar.activation(WT[:, sl], siB[:, 0:1000],
                                             AF.Identity, bias=sjb)
                    else:
                        nc.vector.tensor_scalar(WT[:, sl], siB[:, 0:1000], sjb,
                                                None, op0=ALU.add)
                    nc.vector.scalar_tensor_tensor(
                        WT[:, sl], WT[:, sl], NEG, WT[:, sl],
                        op0=ALU.mult, op1=ALU.max)
                for t in range(8):
                    sl = slice(t * 1000, (t + 1) * 1000)
                    nc.vector.tensor_tensor(WT[:, sl], WT[:, sl], lnc[:, sl],
                                            op=ALU.add)
                for t in range(8):
                    sl = slice(t * 1000, (t + 1) * 1000)
                    nc.scalar.activation(WT[:, sl], WT[:, sl], AF.Exp)
                # den row -> broadcast -> fast reciprocal (DVE custom op)
                denR = pg.tile([2, 1024], F16, tag="denR")
                for hf in range(2):
                    pd = psS.tile([2, CH], F32, tag="S")
                    for t in range(8):
                        kt = 128 if t < 7 else 104
                        nc.tensor.matmul(
                            pd[0:1, :], wp[0:kt, W_ONE:W_ONE + 1],
                            WT[0:kt, t * 1000 + hf * CH:t * 1000 + hf * CH + CH],
                            start=(t == 0), stop=(t == 7))
                    nc.vector.tensor_copy(denR[0:1, hf * CH:hf * CH + CH],
                                          pd[0:1, :])
                rdenB = pg.tile([128, 1024], F32, tag="rdenB")
                for hf in range(2):
                    pr = psA.tile([128, CH], F32, tag="A")
                    nc.tensor.matmul(pr[:], wp[0:1, W_ONER:W_ONER + 128],
                                     denR[0:1, hf * CH:hf * CH + CH],
                                     start=True, stop=True)
                    nc.vector.reciprocal_approx_fast(
                        out=rdenB[:, hf * CH:hf * CH + CH], in_=pr[:])
                # agg = (xnm @ W) * rdenB
                for hf in range(2):
                    pa = psA.tile([128, CH], F32, tag="A")
                    for t in range(8):
                        kt = 128 if t < 7 else 104
                        nc.tensor.matmul(
                            pa[:], xnm[0:kt, (g * 8 + t) * 128:
                                       (g * 8 + t) * 128 + 128],
                            WT[0:kt, t * 1000 + hf * CH:t * 1000 + hf * CH + CH],
                            start=(t == 0), stop=(t == 7))
                    nc.vector.scalar_tensor_tensor(
                        aggT[:, g * 1000 + hf * CH:g * 1000 + hf * CH + CH],
                        pa[:], 1.0, rdenB[:, hf * CH:hf * CH + CH],
                        op0=ALU.mult, op1=ALU.mult)
                # BN partial sums (square on gpsimd, reduces on DVE)
                asl = aggT[:, g * 1000:(g + 1) * 1000]
                nc.vector.tensor_reduce(sumP[:, g:g + 1], asl,
                                        axis=mybir.AxisListType.X, op=ALU.add)
                sqs = pg.tile([128, 1000], F16, tag="sqs")
                nc.gpsimd.tensor_tensor(sqs[:], asl, asl, op=ALU.mult)
                nc.vector.tensor_reduce(sqP[:, g:g + 1], sqs[:],
                                        axis=mybir.AxisListType.X, op=ALU.add)

            # ---- BN stats (gnn_bias folded analytically) + AllReduce
            stats = sm.tile([128, 2], F32)
            s1u = sm.tile([128, 4], F32)
            nc.vector.tensor_reduce(s1u[:, 0:1], sumP[:],
                                    axis=mybir.AxisListType.X, op=ALU.add)
            nc.vector.tensor_reduce(s1u[:, 1:2], sqP[:],
                                    axis=mybir.AxisListType.X, op=ALU.add)
            gb = bias(B_GNN)
            nc.vector.tensor_scalar(s1u[:, 2:3], gb, float(B * N), None,
                                    op0=ALU.mult)
            nc.vector.tensor_tensor(stats[:, 0:1], s1u[:, 0:1], s1u[:, 2:3],
                                    op=ALU.add)
            nc.vector.scalar_tensor_tensor(stats[:, 1:2], s1u[:, 0:1], 2.0,
                                           s1u[:, 2:3], op0=ALU.mult, op1=ALU.add)
            nc.vector.tensor_tensor(stats[:, 1:2], stats[:, 1:2], gb, op=ALU.mult)
            nc.vector.tensor_tensor(stats[:, 1:2], stats[:, 1:2], s1u[:, 1:2],
                                    op=ALU.add)
            nc.sync.dma_start(cc_in, stats[:])
            nc.gpsimd.collective_compute(
                "AllReduce", ALU.add,
                replica_groups=[list(range(n_cores))],
                ins=[cc_in], outs=[cc_out])
            gstats = sm.tile([128, 2], F32)
            nc.sync.dma_start(gstats[:], cc_out)

            # ---- BN coefficients A, Bv
            cf = sm.tile([128, 8], F32)
            mu, msq, var, rsd, A_, Bv = (cf[:, i:i + 1] for i in range(6))
            inv_n = 1.0 / (B * N)
            nc.vector.tensor_scalar_mul(mu, gstats[:, 0:1], inv_n)
            nc.vector.tensor_scalar_mul(msq, gstats[:, 1:2], inv_n)
            nc.vector.tensor_tensor(var, mu, mu, op=ALU.mult)
            nc.vector.tensor_sub(var, msq, var)
            nc.scalar.activation(var, var, AF.Sqrt, bias=bias(B_EPS))
            nc.vector.reciprocal(rsd, var)
            nc.vector.tensor_tensor(A_, bias(B_GAM), rsd, op=ALU.mult)
            nc.vector.tensor_tensor(Bv, mu, A_, op=ALU.mult)
            nc.vector.tensor_sub(Bv, bias(B_BET), Bv)
            nc.vector.tensor_tensor(cf[:, 6:7], bias(B_GNN), A_, op=ALU.mult)
            nc.vector.tensor_tensor(Bv, Bv, cf[:, 6:7], op=ALU.add)

            # ---- s_out = relu(A*agg + Bv) (in place; split ACT/DVE)
            HF2 = NG // 2
            nc.scalar.activation(aggT[:, 0:HF2], aggT[:, 0:HF2], AF.Relu,
                                 bias=Bv, scale=A_)
            nc.vector.tensor_scalar(aggT[:, HF2:NG], aggT[:, HF2:NG], A_, Bv,
                                    op0=ALU.mult, op1=ALU.add)
            nc.vector.tensor_scalar_max(aggT[:, HF2:NG], aggT[:, HF2:NG], 0.0)

            # ---- fusion MLP + out layer
            hT = big.tile([128, NG], F16, tag="s16a")
            for h in range(NH):
                s = h * CH
                ps = psA.tile([128, CH], F32, tag="A")
                nc.tensor.matmul(ps[:], wp[:, W_F1:W_F1 + 128],
                                 aggT[:, s:s + CH], start=True, stop=False)
                nc.tensor.matmul(ps[:], wp[:, W_F1 + 128:W_F1 + 256],
                                 tT[:, s:s + CH], start=False, stop=True)
                if h % 2 == 0:
                    nc.scalar.activation(hT[:, s:s + CH], ps[:], AF.Relu,
                                         bias=bias(B_FB1))
                else:
                    nc.vector.tensor_scalar(hT[:, s:s + CH], ps[:], bias(B_FB1),
                                            None, op0=ALU.add)
                    nc.vector.tensor_scalar_max(hT[:, s:s + CH], hT[:, s:s + CH],
                                                0.0)
            # composite head: c = f_w2 @ out_w ; cb = <out_w, f_b2> + out_b
            cvec = sm.tile([128, 2], F16)
            cb = sm.tile([1, 2], F32)
            nc.vector.tensor_copy(cvec[:, 1:2], bias(B_FB2))  # f_b2 -> f16
            pc = psS.tile([2, CH], F32, tag="S")
            nc.tensor.matmul(pc[0:1, 0:1], cvec[:, 1:2],
                             wp[:, W_OUT:W_OUT + 1], start=True, stop=True)
            pc2 = psX.tile([128, 128], F32, tag="X")
            nc.tensor.matmul(pc2[:, 0:1], wp[:, W_F2P:W_F2P + 128],
                             wp[:, W_OUT:W_OUT + 1], start=True, stop=True)
            nc.vector.tensor_copy(cvec[:, 0:1], pc2[:, 0:1])
            nc.vector.tensor_copy(cb[:, 0:1], pc[0:1, 0:1])
            nc.vector.tensor_tensor(cb[:, 1:2], cb[:, 0:1], ob[:], op=ALU.add)
            for h in range(NH):
                s = h * CH
                ps = psS.tile([2, CH], F32, tag="S")
                nc.tensor.matmul(ps[0:1, :], cvec[:, 0:1],
                                 hT[:, s:s + CH], start=True, stop=True)
                yst = stg.tile([2, CH], F32, tag="ys")
                nc.scalar.activation(yst[0:1, :], ps[0:1, :],
                                     AF.Identity, bias=cb[:, 1:2])
                nc.sync.dma_start(y_out[:, s:s + CH], yst[0:1, :])

    nc.compile()
    return nc


# ---------------------------------------------------------------- entry point
def _prepare(inputs):
    """Returns (nc, in_maps) — host prep + cached module build."""
    data = np.asarray(inputs["data"], np.float32)
    edge_index = np.asarray(inputs["edge_index"])

    pre = _prep_indices(edge_index)
    Kout = pre["Kout"]

    key = (Kout,)
    if key not in _CACHE:
        _CACHE[key] = _build(Kout, M)
    nc = _CACHE[key]

    f16 = np.float16

    def t16(a):  # transpose [r, c] -> [c, r] f16 contiguous
        return np.ascontiguousarray(np.asarray(a, np.float32).T).astype(f16)

    wpack = np.zeros((128, 1184), f16)
    wpack[:, 0:128] = t16(inputs["lin_w"])
    wpack[:, 128:256] = t16(inputs["v_w"])
    wpack[:, 256:384] = t16(inputs["f_w2"])
    wpack[:, 384:640] = np.ascontiguousarray(
        np.asarray(inputs["f_w1"], np.float32).T).astype(f16).reshape(2, 128, 128
        ).transpose(1, 0, 2).reshape(128, 256)
    wpack[:, 640] = np.asarray(inputs["att_i"], np.float32).astype(f16)
    wpack[:, 641] = np.asarray(inputs["att_j"], np.float32).astype(f16)
    wpack[:, 642] = np.asarray(inputs["att_em_i"], np.float32).astype(f16)
    wpack[:, 643] = np.asarray(inputs["att_em_j"], np.float32).astype(f16)
    wpack[:, 644] = np.asarray(inputs["out_w"], np.float32)[0].astype(f16)
    wpack[:, 645:773] = np.eye(128, dtype=f16)
    wpack[:, 773] = 1.0
    wpack[0, 774:902] = 1.0
    wpack[:, 902:1030] = np.asarray(inputs["f_w2"], np.float32).astype(f16)
    wpack[:, 1030:1158] = np.asarray(inputs["lin_w"], np.float32).astype(f16)

    bpack = np.zeros((128, 8), np.float32)
    bpack[:, 0] = np.asarray(inputs["v_b"], np.float32)
    bpack[:, 1] = np.asarray(inputs["gnn_bias"], np.float32)
    bpack[:, 2] = np.asarray(inputs["f_b1"], np.float32)
    bpack[:, 3] = np.asarray(inputs["f_b2"], np.float32)
    bpack[:, 4] = np.asarray(inputs["bn_gamma"], np.float32)
    bpack[:, 5] = np.asarray(inputs["bn_beta"], np.float32)
    bpack[:, 6] = EPS
    outb = np.asarray(inputs["out_b"], np.float32).reshape(1, 1)

    embT = t16(inputs["emb"])

    shared = dict(
        embT=embT, wpack=wpack, bpack=bpack, outb=outb,
        sidx2=pre["sidx2"], cntw=pre["cntw"],
    )
    in_maps = []
    for d in range(M):
        x0T = np.ascontiguousarray(
            data[d * G:(d + 1) * G].transpose(2, 0, 1).reshape(128, NG)
        ).astype(f16)
        in_maps.append(dict(shared, x0T=x0T))
    return nc, in_maps


def kernel(**inputs):
    nc, in_maps = _prepare(inputs)
    res = run_bass_kernel_spmd(nc, in_maps, list(range(M)))
    out = np.empty(B * N, np.float32)
    for d in range(M):
        out[d * NG:(d + 1) * NG] = res.results[d]["y"].reshape(-1)
    return out


# revision 39
# speedup vs baseline: 1.1324x; 1.0002x over previous
"""EnhancedGDN Trainium2 kernel (v3 — gather-free).

Data-parallel over batch B=64 across 8 NeuronCores (8 graphs each).
All 64 graphs share one edge list; the only gpsimd work is ONE shared
cnt-mask scatter (ap_gather is ~36us/call on HW — avoided entirely).

Per device (8 graphs):
  t_out = data @ v_w.T + v_b              (temporal attn: window=1)
  x     = data @ lin_w.T                  (xnm tiles, node-major)
  s_i/s_j per node from att vectors (+ tiled emb scores)
  lnc   = Ln(scatter(cnt))                (once; ln(0)=-inf kills non-edges)
  per graph g:
    alpha[j,i] = s_j[g,j] + s_i[g,i]      (DVE: rank-1 siB + per-partition sj)
    W = exp(lrelu(alpha) + lnc)           (dense [src, dst], f16)
    den = ones @ W ; rdenB = recip(ones x den)   (ACT table recip)
    agg = (xnm @ W) * rdenB               (normalize on evict)
  BatchNorm over all 64k nodes (AllReduce of sums) + ReLU
  out   = (relu([s_out|t_out] @ f_w1.T + f_b1) @ f_w2.T + f_b2) @ out_w.T + out_b
"""

import os

os.environ.setdefault("NEURON_RT_RESET_CORES", "1")

import numpy as np

import concourse.bass as bass
import concourse.bacc as bacc
import concourse.tile as tile
from concourse import mybir
from concourse.bass_utils import run_bass_kernel_spmd

B, N, D, E = 64, 1000, 128, 20000
M = 8          # devices
G = B // M     # graphs per device
NG = G * N     # nodes per device
NEG = 0.2
EPS = 1e-5

F16 = mybir.dt.float16
F32 = mybir.dt.float32
I16 = mybir.dt.int16
AF = mybir.ActivationFunctionType
ALU = mybir.AluOpType

_CACHE = {}


# ---------------------------------------------------------------- host index prep
def _prep_indices(edge_index):
    src = edge_index[0].astype(np.int64)
    dst = edge_index[1].astype(np.int64)
    key = dst * N + src
    uniq, cnt = np.unique(key, return_counts=True)
    ii = uniq // N
    jj = uniq % N
    # add self loops (reference removes none exist, then adds them)
    ii = np.concatenate([ii, np.arange(N)])
    jj = np.concatenate([jj, np.arange(N)])
    cc = np.concatenate([cnt, np.ones(N, np.int64)]).astype(np.float32)

    # out-CSR grouped by src j: scatter indices + cnt values, [j, (c, k)]
    order = np.argsort(jj, kind="stable")
    oj, oi, oc = jj[order], ii[order], cc[order]
    outdeg = np.bincount(oj, minlength=N)
    Kout = max(4, (int(outdeg.max()) + 3) // 4 * 4)
    out_i_sc = np.full((1024, Kout), -1, np.int64)  # -1 pads ignored by scatter
    out_c = np.zeros((1024, Kout), np.float32)
    starts = np.searchsorted(oj, np.arange(N))
    kpos = np.arange(len(oj)) - starts[oj]
    out_i_sc[oj, kpos] = oi
    out_c[oj, kpos] = oc

    scat = out_i_sc.reshape(8, 128, Kout).transpose(1, 0, 2)      # [j, c, k]
    sidx2 = np.ascontiguousarray(scat.reshape(128, 8 * Kout)).astype(np.int16)
    cvals = out_c.reshape(8, 128, Kout).transpose(1, 0, 2)
    cntw = np.ascontiguousarray(cvals.reshape(128, 8 * Kout)).astype(np.float16)
    return dict(Kout=Kout, sidx2=sidx2, cntw=cntw)


# ---------------------------------------------------------------- device module
def _build(Kout, n_cores):
    NH = 16             # chunks of 500 over 8000
    CH = 500

    nc = bacc.Bacc("TRN2", target_bir_lowering=False, debug=False,
                   num_devices=n_cores)

    def din(name, shape, dt):
        return nc.dram_tensor(name, shape, dt, kind="ExternalInput").ap()

    x0T = din("x0T", [128, NG], F16)
    embT = din("embT", [128, N], F16)
    wpack = din("wpack", [128, 1184], F16)
    bpack = din("bpack", [128, 8], F32)
    outb = din("outb", [1, 1], F32)
    sidx2_d = din("sidx2", [128, 8 * Kout], I16)
    cntw_d = din("cntw", [128, 8 * Kout], F16)
    y_out = nc.dram_tensor("y", [1, NG], F32, kind="ExternalOutput").ap()

    cc_in = nc.dram_tensor("cc_in", [128, 2], F32).ap()
    cc_out = nc.dram_tensor("cc_out", [128, 2], F32, addr_space="Shared").ap()
    cc_win = nc.dram_tensor("cc_win", [128, 2], F32).ap()
    cc_wout = nc.dram_tensor("cc_wout", [128, 2], F32, addr_space="Shared").ap()

    # wpack column layout
    W_LIN, W_V, W_F2, W_F1, W_ATTQ, W_ATTEM, W_OUT, W_ID, W_ONE = (
        0, 128, 256, 384, 640, 642, 644, 645, 773)
    W_ONER = 774
    W_F2P = 902
    W_LINP = 1030
    # bpack columns: v_b, gnn_bias, f_b1, f_b2, bn_gamma, bn_beta
    B_VB, B_GNN, B_FB1, B_FB2, B_GAM, B_BET, B_EPS = 0, 1, 2, 3, 4, 5, 6

    with tile.TileContext(nc) as tc:
        with (
            tc.tile_pool(name="cst", bufs=1) as cst,
            tc.tile_pool(name="big", bufs=1) as big,
            tc.tile_pool(name="wt", bufs=2) as wtp,
            tc.tile_pool(name="sm", bufs=1) as sm,
            tc.tile_pool(name="pg", bufs=2) as pg,
            tc.tile_pool(name="stg", bufs=4) as stg,
            tc.tile_pool(name="psS", bufs=2, space="PSUM") as psS,
            tc.tile_pool(name="psX", bufs=2, space="PSUM") as psX,
            tc.tile_pool(name="psA", bufs=4, space="PSUM") as psA,
        ):
            # ---- load constants (x0 first: biggest + on critical path)
            x0 = big.tile([128, NG], F16, tag="s16a")
            nc.sync.dma_start(x0[:], x0T)
            wp = cst.tile([128, 1184], F16)
            nc.sync.dma_start(wp[:], wpack)
            bp = cst.tile([128, 8], F32)
            nc.sync.dma_start(bp[:], bpack)
            ob = cst.tile([1, 1], F32)
            nc.sync.dma_start(ob[:], outb)
            emb = cst.tile([128, N], F16)
            nc.sync.dma_start(emb[:], embT)
            sidx = cst.tile([128, 8 * Kout], I16)
            nc.sync.dma_start(sidx[:], sidx2_d)
            cntw = cst.tile([128, 8 * Kout], F16)
            nc.sync.dma_start(cntw[:], cntw_d)

            def bias(col):
                return bp[:, col:col + 1]

            # warm up the collective path early (absorbs setup/skew)
            warm = sm.tile([128, 2], F32)
            nc.vector.memset(warm[:], 0.0)
            nc.sync.dma_start(cc_win, warm[:])
            nc.gpsimd.collective_compute(
                "AllReduce", ALU.add,
                replica_groups=[list(range(n_cores))],
                ins=[cc_win], outs=[cc_wout])

            # ---- shared cnt mask (scatter once; zeros kill non-edges)
            cmask = big.tile([128, NG], F16, tag="lnc")
            for t in range(8):
                nc.gpsimd.local_scatter(
                    cmask[:, t * 1000:(t + 1) * 1000],
                    cntw[:, t * Kout:(t + 1) * Kout],
                    sidx[:, t * Kout:(t + 1) * Kout],
                    channels=128, num_elems=N, num_idxs=Kout)

            # ---- node scores: attc = lin^T attq ; emsc = att_em @ emb
            attc = sm.tile([128, 2], F16)
            pat = psX.tile([128, 128], F32, tag="X")
            nc.tensor.matmul(pat[:, 0:2], wp[:, W_LINP:W_LINP + 128],
                             wp[:, W_ATTQ:W_ATTQ + 2], start=True, stop=True)
            nc.vector.tensor_copy(attc[:], pat[:, 0:2])
            emsc = sm.tile([2, N], F32)
            for h in range(2):
                ps = psS.tile([2, CH], F32, tag="S")
                nc.tensor.matmul(ps[:], wp[:, W_ATTEM:W_ATTEM + 2],
                                 emb[:, h * CH:(h + 1) * CH], start=True, stop=True)
                nc.vector.tensor_copy(emsc[:, h * CH:(h + 1) * CH], ps[:])

            # scores -> sNN [16, 1024] f16 (rows 0-7 s_i[g], rows 8-15 s_j[g])
            sNN = sm.tile([16, 1024], F16)
            nc.vector.memset(sNN[:], 0.0)
            for h in range(NH):
                s = h * CH
                g, off = divmod(s, 1000)
                ps = psS.tile([2, CH], F32, tag="S")
                nc.tensor.matmul(ps[:], attc[:, 0:2],
                                 x0[:, s:s + CH], start=True, stop=True)
                st = stg.tile([2, CH], F16, tag="sc")
                nc.vector.tensor_tensor(st[:], ps[:], emsc[:, off:off + CH],
                                        op=ALU.add)
                nc.sync.dma_start(
                    sNN[:, off:off + CH].rearrange("(a g) f -> g a f", a=2)[g],
                    st[:, :])

            # ---- exp factors: exp(lrelu(a)) == max(e^a, e^{0.2a}) and
            # e^{si+sj} factors into rank-1 products of per-node exponentials
            sNNe = sm.tile([16, 1024], F16)
            sNNf = sm.tile([16, 1024], F16)
            nc.scalar.activation(sNNe[:], sNN[:], AF.Exp)
            nc.scalar.activation(sNNf[:], sNN[:], AF.Exp, scale=NEG)
            # sAB[j, 16t+r] = e^{sNN[r, 128t+j]} (transposed col tables)
            sAB = sm.tile([128, 128], F16)
            sABe = sm.tile([128, 128], F32)
            sABf = sm.tile([128, 128], F32)
            for t in range(8):
                pt = psX.tile([128, 128], F16, tag="X")
                nc.tensor.transpose(pt[:, 0:16], sNN[:, t * 128:(t + 1) * 128],
                                    wp[0:16, W_ID:W_ID + 16])
                nc.vector.tensor_copy(sAB[:, t * 16:(t + 1) * 16], pt[:, 0:16])
            nc.scalar.activation(sABe[:], sAB[:], AF.Exp)
            nc.scalar.activation(sABf[:], sAB[:], AF.Exp, scale=NEG)
            # row-0 staging of e^{si[g]} / e^{0.2 si[g]} for rank-1 rhs
            siRE = sm.tile([1, 8 * 1024], F16)
            siRF = sm.tile([1, 8 * 1024], F16)
            for g in range(G):
                nc.sync.dma_start(siRE[0:1, g * 1024:g * 1024 + 1024],
                                  sNNe[g:g + 1, :])
                nc.sync.dma_start(siRF[0:1, g * 1024:g * 1024 + 1024],
                                  sNNf[g:g + 1, :])

            # ---- xnm tiles (x in node-major, direct from x0) + tT (off path)
            xnm = big.tile([128, 64 * 128], F16, tag="xnm")
            for g in range(G):
                for t in range(8):
                    w = 128 if t < 7 else 104
                    n0 = g * 1000 + t * 128
                    px = psX.tile([128, 128], F32, tag="X")
                    nc.tensor.matmul(px[0:w, :], x0[:, n0:n0 + w],
                                     wp[:, W_LIN:W_LIN + 128], start=True,
                                     stop=True)
                    dst = xnm[0:w, (g * 8 + t) * 128:(g * 8 + t) * 128 + 128]
                    if t % 2 == 0:
                        nc.scalar.activation(dst, px[0:w, :], AF.Identity)
                    else:
                        nc.vector.tensor_copy(dst, px[0:w, :])
            tT = big.tile([128, NG], F16, tag="tT")
            for h in range(NH):
                s = h * CH
                ps2 = psA.tile([128, CH], F32, tag="A")
                nc.tensor.matmul(ps2[:], wp[:, W_V:W_V + 128],
                                 x0[:, s:s + CH], start=True, stop=True)
                if h % 2 == 0:
                    nc.scalar.activation(tT[:, s:s + CH], ps2[:], AF.Identity,
                                         bias=bias(B_VB))
                else:
                    nc.vector.tensor_scalar(tT[:, s:s + CH], ps2[:], bias(B_VB),
                                            None, op0=ALU.add)

            # ---- per-graph: dense W via exp-factor max trick, den+agg matmuls
            # software-pipelined: W(g) is built while W(g-1) is consumed
            aggT = big.tile([128, NG], F16, tag="agg")
            sumP = sm.tile([128, 16], F32)
            sqP = sm.tile([128, 8], F32)

            def build_w(g):
                # EiB/FiB = ones (x) e^{si}/e^{0.2 si}  [128, 1000]
                eib = pg.tile([128, 1024], F16, tag="eib")
                fib = pg.tile([128, 1024], F16, tag="fib")
                for hf in range(2):
                    pe = psA.tile([128, CH], F32, tag="A")
                    nc.tensor.matmul(pe[:], wp[0:1, W_ONER:W_ONER + 128],
                                     siRE[0:1, g * 1024 + hf * CH:
                                          g * 1024 + hf * CH + CH],
                                     start=True, stop=True)
                    if hf == 0:
                        nc.vector.tensor_copy(eib[:, 0:CH], pe[:])
                    else:
                        nc.scalar.activation(eib[:, CH:2 * CH], pe[:],
                                             AF.Identity)
                    pf = psA.tile([128, CH], F32, tag="A")
                    nc.tensor.matmul(pf[:], wp[0:1, W_ONER:W_ONER + 128],
                                     siRF[0:1, g * 1024 + hf * CH:
                                          g * 1024 + hf * CH + CH],
                                     start=True, stop=True)
                    if hf == 0:
                        nc.scalar.activation(fib[:, 0:CH], pf[:], AF.Identity)
                    else:
                        nc.vector.tensor_copy(fib[:, CH:2 * CH], pf[:])
                # W[j,i] = cnt * max(Ej*Ei, Fj*Fi)
                WT = wtp.tile([128, 8000], F16, tag="wt")
                for t in range(8):
                    sl = slice(t * 1000, (t + 1) * 1000)
                    col = 16 * t + 8 + g
                    tmp = pg.tile([128, 1000], F16, tag="tmp", bufs=4)
                    nc.scalar.activation(tmp[:], eib[:, 0:1000], AF.Identity,
                                         scale=sABe[:, col:col + 1])
                    nc.vector.scalar_tensor_tensor(
                        tmp[:], fib[:, 0:1000], sABf[:, col:col + 1], tmp[:],
                        op0=ALU.mult, op1=ALU.max)
                    eng = nc.vector if t % 2 == 0 else nc.gpsimd
                    eng.tensor_tensor(WT[:, sl], tmp[:], cmask[:, sl],
                                      op=ALU.mult)
                return WT

            def consume_w(g, WT):
                # den row -> broadcast -> fast reciprocal (DVE custom op)
                denR = pg.tile([2, 1024], F16, tag="denR")
                for hf in range(2):
                    pd = psS.tile([2, CH], F32, tag="S")
                    for t in range(8):
                        kt = 128 if t < 7 else 104
                        nc.tensor.matmul(
                            pd[0:1, :], wp[0:kt, W_ONE:W_ONE + 1],
                            WT[0:kt, t * 1000 + hf * CH:t * 1000 + hf * CH + CH],
                            start=(t == 0), stop=(t == 7))
                    nc.vector.tensor_copy(denR[0:1, hf * CH:hf * CH + CH],
                                          pd[0:1, :])
                rdenB = pg.tile([128, 1024], F32, tag="rdenB")
                for hf in range(2):
                    pr = psA.tile([128, CH], F32, tag="A")
                    nc.tensor.matmul(pr[:], wp[0:1, W_ONER:W_ONER + 128],
                                     denR[0:1, hf * CH:hf * CH + CH],
                                     start=True, stop=True)
                    nc.vector.reciprocal_approx_fast(
                        out=rdenB[:, hf * CH:hf * CH + CH], in_=pr[:])
                # agg = (xnm @ W) * rdenB ; accum_out gives BN sums for free
                for hf in range(2):
                    pa = psA.tile([128, CH], F32, tag="A")
                    for t in range(8):
                        kt = 128 if t < 7 else 104
                        nc.tensor.matmul(
                            pa[:], xnm[0:kt, (g * 8 + t) * 128:
                                       (g * 8 + t) * 128 + 128],
                            WT[0:kt, t * 1000 + hf * CH:t * 1000 + hf * CH + CH],
                            start=(t == 0), stop=(t == 7))
                    nc.vector.scalar_tensor_tensor(
                        aggT[:, g * 1000 + hf * CH:g * 1000 + hf * CH + CH],
                        pa[:], 1.0, rdenB[:, hf * CH:hf * CH + CH],
                        op0=ALU.mult, op1=ALU.mult,
                        accum_out=sumP[:, 2 * g + hf:2 * g + hf + 1])
                # BN square sums (square on gpsimd, reduce on DVE)
                asl = aggT[:, g * 1000:(g + 1) * 1000]
                sqs = pg.tile([128, 1000], F16, tag="sqs")
                nc.gpsimd.tensor_tensor(sqs[:], asl, asl, op=ALU.mult)
                nc.vector.tensor_reduce(sqP[:, g:g + 1], sqs[:],
                                        axis=mybir.AxisListType.X, op=ALU.add)

            prev = build_w(0)
            for g in range(1, G):
                cur = build_w(g)
                consume_w(g - 1, prev)
                prev = cur
            consume_w(G - 1, prev)

            # ---- BN stats (gnn_bias folded analytically) + AllReduce
            stats = sm.tile([128, 2], F32)
            s1u = sm.tile([128, 4], F32)
            nc.vector.tensor_reduce(s1u[:, 0:1], sumP[:],
                                    axis=mybir.AxisListType.X, op=ALU.add)
            nc.vector.tensor_reduce(s1u[:, 1:2], sqP[:],
                                    axis=mybir.AxisListType.X, op=ALU.add)
            gb = bias(B_GNN)
            nc.vector.tensor_scalar(s1u[:, 2:3], gb, float(B * N), None,
                                    op0=ALU.mult)
            nc.vector.tensor_tensor(stats[:, 0:1], s1u[:, 0:1], s1u[:, 2:3],
                                    op=ALU.add)
            nc.vector.scalar_tensor_tensor(stats[:, 1:2], s1u[:, 0:1], 2.0,
                                           s1u[:, 2:3], op0=ALU.mult, op1=ALU.add)
            nc.vector.tensor_tensor(stats[:, 1:2], stats[:, 1:2], gb, op=ALU.mult)
            nc.vector.tensor_tensor(stats[:, 1:2], stats[:, 1:2], s1u[:, 1:2],
                                    op=ALU.add)
            nc.sync.dma_start(cc_in, stats[:])
            nc.gpsimd.collective_compute(
                "AllReduce", ALU.add,
                replica_groups=[list(range(n_cores))],
                ins=[cc_in], outs=[cc_out])
            gstats = sm.tile([128, 2], F32)
            nc.sync.dma_start(gstats[:], cc_out)

            # ---- BN coefficients A, Bv
            cf = sm.tile([128, 8], F32)
            mu, msq, var, rsd, A_, Bv = (cf[:, i:i + 1] for i in range(6))
            inv_n = 1.0 / (B * N)
            nc.vector.tensor_scalar_mul(mu, gstats[:, 0:1], inv_n)
            nc.vector.tensor_scalar_mul(msq, gstats[:, 1:2], inv_n)
            nc.vector.tensor_tensor(var, mu, mu, op=ALU.mult)
            nc.vector.tensor_sub(var, msq, var)
            nc.scalar.activation(var, var, AF.Sqrt, bias=bias(B_EPS))
            nc.vector.reciprocal(rsd, var)
            nc.vector.tensor_tensor(A_, bias(B_GAM), rsd, op=ALU.mult)
            nc.vector.tensor_tensor(Bv, mu, A_, op=ALU.mult)
            nc.vector.tensor_sub(Bv, bias(B_BET), Bv)
            nc.vector.tensor_tensor(cf[:, 6:7], bias(B_GNN), A_, op=ALU.mult)
            nc.vector.tensor_tensor(Bv, Bv, cf[:, 6:7], op=ALU.add)

            # ---- s_out = relu(A*agg + Bv) (in place; split ACT/DVE)
            HF2 = NG // 2
            nc.scalar.activation(aggT[:, 0:HF2], aggT[:, 0:HF2], AF.Relu,
                                 bias=Bv, scale=A_)
            nc.vector.tensor_scalar(aggT[:, HF2:NG], aggT[:, HF2:NG], A_, Bv,
                                    op0=ALU.mult, op1=ALU.add)
            nc.vector.tensor_scalar_max(aggT[:, HF2:NG], aggT[:, HF2:NG], 0.0)

            # ---- fusion MLP + out layer
            hT = big.tile([128, NG], F16, tag="s16a")
            for h in range(NH):
                s = h * CH
                ps = psA.tile([128, CH], F32, tag="A")
                nc.tensor.matmul(ps[:], wp[:, W_F1:W_F1 + 128],
                                 aggT[:, s:s + CH], start=True, stop=False)
                nc.tensor.matmul(ps[:], wp[:, W_F1 + 128:W_F1 + 256],
                                 tT[:, s:s + CH], start=False, stop=True)
                if h % 2 == 0:
                    nc.scalar.activation(hT[:, s:s + CH], ps[:], AF.Relu,
                                         bias=bias(B_FB1))
                else:
                    nc.vector.tensor_scalar(hT[:, s:s + CH], ps[:], bias(B_FB1),
                                            None, op0=ALU.add)
                    nc.vector.tensor_scalar_max(hT[:, s:s + CH], hT[:, s:s + CH],
                                                0.0)
            # composite head: c = f_w2 @ out_w ; cb = <out_w, f_b2> + out_b
            cvec = sm.tile([128, 2], F16)
            cb = sm.tile([1, 2], F32)
            nc.vector.tensor_copy(cvec[:, 1:2], bias(B_FB2))  # f_b2 -> f16
            pc = psS.tile([2, CH], F32, tag="S")
            nc.tensor.matmul(pc[0:1, 0:1], cvec[:, 1:2],
                             wp[:, W_OUT:W_OUT + 1], start=True, stop=True)
            pc2 = psX.tile([128, 128], F32, tag="X")
            nc.tensor.matmul(pc2[:, 0:1], wp[:, W_F2P:W_F2P + 128],
                             wp[:, W_OUT:W_OUT + 1], start=True, stop=True)
            nc.vector.tensor_copy(cvec[:, 0:1], pc2[:, 0:1])
            nc.vector.tensor_copy(cb[:, 0:1], pc[0:1, 0:1])
            nc.vector.tensor_tensor(cb[:, 1:2], cb[:, 0:1], ob[:], op=ALU.add)
            for h in range(NH):
                s = h * CH
                ps = psS.tile([2, CH], F32, tag="S")
                nc.tensor.matmul(ps[0:1, :], cvec[:, 0:1],
                                 hT[:, s:s + CH], start=True, stop=True)
                yst = stg.tile([2, CH], F32, tag="ys")
                nc.scalar.activation(yst[0:1, :], ps[0:1, :],
                                     AF.Identity, bias=cb[:, 1:2])
                nc.sync.dma_start(y_out[:, s:s + CH], yst[0:1, :])

    nc.compile()
    return nc


# ---------------------------------------------------------------- entry point
def _prepare(inputs):
    """Returns (nc, in_maps) — host prep + cached module build."""
    data = np.asarray(inputs["data"], np.float32)
    edge_index = np.asarray(inputs["edge_index"])

    pre = _prep_indices(edge_index)
    Kout = pre["Kout"]

    key = (Kout,)
    if key not in _CACHE:
        _CACHE[key] = _build(Kout, M)
    nc = _CACHE[key]

    f16 = np.float16

    def t16(a):  # transpose [r, c] -> [c, r] f16 contiguous
        return np.ascontiguousarray(np.asarray(a, np.float32).T).astype(f16)

    wpack = np.zeros((128, 1184), f16)
    wpack[:, 0:128] = t16(inputs["lin_w"])
    wpack[:, 128:256] = t16(inputs["v_w"])
    wpack[:, 256:384] = t16(inputs["f_w2"])
    wpack[:, 384:640] = np.ascontiguousarray(
        np.asarray(inputs["f_w1"], np.float32).T).astype(f16).reshape(2, 128, 128
        ).transpose(1, 0, 2).reshape(128, 256)
    wpack[:, 640] = np.asarray(inputs["att_i"], np.float32).astype(f16)
    wpack[:, 641] = np.asarray(inputs["att_j"], np.float32).astype(f16)
    wpack[:, 642] = np.asarray(inputs["att_em_i"], np.float32).astype(f16)
    wpack[:, 643] = np.asarray(inputs["att_em_j"], np.float32).astype(f16)
    wpack[:, 644] = np.asarray(inputs["out_w"], np.float32)[0].astype(f16)
    wpack[:, 645:773] = np.eye(128, dtype=f16)
    wpack[:, 773] = 1.0
    wpack[0, 774:902] = 1.0
    wpack[:, 902:1030] = np.asarray(inputs["f_w2"], np.float32).astype(f16)
    wpack[:, 1030:1158] = np.asarray(inputs["lin_w"], np.float32).astype(f16)

    bpack = np.zeros((128, 8), np.float32)
    bpack[:, 0] = np.asarray(inputs["v_b"], np.float32)
    bpack[:, 1] = np.asarray(inputs["gnn_bias"], np.float32)
    bpack[:, 2] = np.asarray(inputs["f_b1"], np.float32)
    bpack[:, 3] = np.asarray(inputs["f_b2"], np.float32)
    bpack[:, 4] = np.asarray(inputs["bn_gamma"], np.float32)
    bpack[:, 5] = np.asarray(inputs["bn_beta"], np.float32)
    bpack[:, 6] = EPS
    outb = np.asarray(inputs["out_b"], np.float32).reshape(1, 1)

    embT = t16(inputs["emb"])

    shared = dict(
        embT=embT, wpack=wpack, bpack=bpack, outb=outb,
        sidx2=pre["sidx2"], cntw=pre["cntw"],
    )
    in_maps = []
    for d in range(M):
        x0T = np.ascontiguousarray(
            data[d * G:(d + 1) * G].transpose(2, 0, 1).reshape(128, NG)
        ).astype(f16)
        in_maps.append(dict(shared, x0T=x0T))
    return nc, in_maps


def kernel(**inputs):
    nc, in_maps = _prepare(inputs)
    res = run_bass_kernel_spmd(nc, in_maps, list(range(M)))
    out = np.empty(B * N, np.float32)
    for d in range(M):
        out[d * NG:(d + 1) * NG] = res.results[d]["y"].reshape(-1)
    return out
